# revision 25
# baseline (speedup 1.0000x reference)
"""GAT layer (gnn_message_passing) on 8 Trainium2 NeuronCores.

v2 strategy (replicated z, zero collectives, dst-owner edge sharding):
  - Every core computes the FULL z = h @ W.T table (h shipped bf16,
    ~25.6MB load + ~45us of bf16 matmul) and writes it to a local DRAM
    table of pure-z 256B rows (identity node->row mapping).  This replaces
    the v1 sharded-z + AllGather (283us of collective on the sim critical
    path) and removes every inter-core sync point.
  - Edges are bucketed by the owner of dst (6250 nodes/core), so segment
    softmax is fully local.  Softmax uses the no-max-subtraction form.
  - Per (core, half) the owned nodes are degree-sorted into a dst-major
    slot layout: partition p = node slot, free column = edge rank.
    dma_gather (int16 indices -> table split in two halves at node 25088)
    fetches z[src] rows straight into that layout.
  - zs[src] per edge is recomputed on the vector engine from the gathered
    bf16 z rows (dot with a_src along the free axis) instead of being
    stored in the table -- keeps gather rows at 256B (vs 512B in v1).
  - zd for owned nodes comes from a tiny per-core matmul over the core's
    own h slice (rhs = w_d column only), permuted to slot order via a
    dma_scatter_add round trip through DRAM (zdperm).
  - Weighted segment-sum runs on the TensorEngine as identity-lhsT matmul
    chains accumulating in PSUM; denominators via a vector reduce.
  - Both halves scatter-add into the SAME 129-col accumulator block
    (scatter_add merges them), then a final divide produces out.
  - Zero-in-degree nodes get a fake self-edge on the host so out == z.
"""

import numpy as np
import ml_dtypes

import concourse.bass as bass
import concourse.mybir as mybir
import concourse.tile as tile
from concourse import bacc
from concourse import library_config
from concourse.bass import ts
from concourse.bass_utils import run_bass_kernel_spmd

F32 = mybir.dt.float32
BF16 = mybir.dt.bfloat16
I16 = mybir.dt.int16

NC = 8          # cores
P = 128         # partitions
IN_DIM = 256
OUT_DIM = 128
KCH = IN_DIM // P       # 2 k-chunks for the feature matmuls
ROW = 128               # bf16 elems per table row (256 B, pure z)
ACC2 = 192              # f32 elems per accumulator row (768 B = 3*256)
SCAT_ELEMS = 129        # f32 elems scattered per slot ([agg128 | den])
ZDP_STRIDE = 64         # f32 stride of the zd permute buffer (256 B)
CHUNK_COLS = 32         # max gather columns per chunk
HCHUNK = 6272           # h columns per phase-1 SBUF chunk (49 tiles; 4 chunks = table0)
GRP = 3                 # tiles per phase-1 PSUM group (3 PSUM banks)


class Cfg:
    def __init__(self, n_nodes, n_edges):
        assert n_nodes % NC == 0
        self.N = n_nodes
        self.E = n_edges
        self.NPC = n_nodes // NC                  # 6250 owned nodes/core
        self.NPAD = ((self.NPC + P - 1) // P) * P  # 6400
        self.NT = self.NPAD // P                   # 50 owned tiles
        self.NF = ((n_nodes + P - 1) // P) * P     # 50048 full-table rows
        self.NTF = self.NF // P                    # 391 full tiles
        # table halves split at a tile boundary so int16 gather indices fit
        self.HALF = (self.NTF // 2 + 1) * P        # 25088
        self.HB = [0, self.HALF]
        self.HROWS = [self.HALF, self.NF - self.HALF]
        assert max(self.HROWS) < 32768


def _wrap16(flat, dtype=np.int16):
    """flat[i] -> [128, len/16] with flat[i] at [i%16, i//16], replicated x8."""
    n = flat.shape[0]
    assert n % 16 == 0
    w = flat.reshape(n // 16, 16).T.astype(dtype)  # [16, n/16]
    return np.tile(w, (8, 1))


def host_prep(cfg, src, dst):
    """Build the common tile structure + per-core index/mask arrays."""
    N, NPC, NPAD, HALF = cfg.N, cfg.NPC, cfg.NPAD, cfg.HALF
    src = np.asarray(src, np.int64).copy()
    dst = np.asarray(dst, np.int64).copy()

    # fake self-edges for isolated (zero in-degree) nodes -> out == z exactly
    deg_tot = np.bincount(dst, minlength=N)
    iso = np.nonzero(deg_tot == 0)[0]
    if iso.size:
        src = np.concatenate([src, iso])
        dst = np.concatenate([dst, iso])

    owner = dst // NPC
    halves = (src >= HALF).astype(np.int64)
    table_local = src - halves * HALF            # identity node layout
    assert table_local.max() < 32768 and table_local.min() >= 0

    per = {}
    deg_sorted_all = []
    for c in range(NC):
        for h in (0, 1):
            m = (owner == c) & (halves == h)
            es = table_local[m]               # gather index of each edge
            ed = dst[m] - c * NPC             # local dst node
            deg = np.bincount(ed, minlength=NPAD)
            order = np.argsort(-deg, kind="stable")  # node_of_slot [NPAD]
            sL = np.empty(NPAD, np.int64)
            sL[order] = np.arange(NPAD)              # slot of node
            per[(c, h)] = dict(es=es, ed=ed, deg=deg, order=order, sL=sL)
            deg_sorted_all.append(deg[order])

    # common tile widths
    NT = cfg.NT
    W = np.zeros(NT, np.int64)
    for dsrt in deg_sorted_all:
        W = np.maximum(W, dsrt[::P][:NT])
    NTp = int(np.nonzero(W > 0)[0][-1]) + 1 if (W > 0).any() else 0
    W = W[:NTp]
    colstart = np.concatenate([[0], np.cumsum(W)]).astype(np.int64)
    C = int(colstart[-1])
    CP = C  # gather cols total per half

    # chunks: runs of equal W, capped at CHUNK_COLS columns
    chunks = []  # (t0, nt, W)
    t = 0
    while t < NTp:
        w = int(W[t])
        nt = 1
        while (
            t + nt < NTp
            and int(W[t + nt]) == w
            and (nt + 1) * w <= CHUNK_COLS
        ):
            nt += 1
        chunks.append((t, nt, w))
        t += nt

    # per-(core,half) flat arrays
    data = {}
    for c in range(NC):
        for h in (0, 1):
            d = per[(c, h)]
            es, ed, deg, order, sL = (
                d["es"], d["ed"], d["deg"], d["order"], d["sL"],
            )
            slot = sL[ed]
            tile_of = slot // P
            part_of = slot % P
            # rank of each edge within its dst node
            o = np.argsort(slot, kind="stable")
            slot_s = slot[o]
            es_s = es[o]
            tile_s = tile_of[o]
            part_s = part_of[o]
            counts = np.bincount(slot_s, minlength=NPAD)
            starts = np.concatenate([[0], np.cumsum(counts)])[:-1]
            rank = np.arange(slot_s.size) - starts[slot_s]
            keep = tile_s < NTp
            assert keep.all(), "edge landed outside processed tiles"
            cglob = colstart[tile_s] + rank
            assert (rank < W[tile_s]).all()
            pos = cglob * P + part_s

            flat_idx = np.zeros(CP * P, np.int16)
            flat_idx[pos] = es_s.astype(np.int16)
            mask = np.zeros((P, CP), np.float32)
            mask[part_s, cglob] = 1.0

            data[(c, h)] = dict(
                gidx=_wrap16(flat_idx),
                gmask=mask,
                mscat=_wrap16(order[: NTp * P].astype(np.int16)),
                zdscat=_wrap16(sL.astype(np.int16)),
            )

    struct = dict(W=W, NTp=NTp, colstart=colstart, C=C, chunks=chunks)
    return struct, data


def build_program(cfg, struct, stage=5, sub=4):
    NPAD, NT, NF, NTF = cfg.NPAD, cfg.NT, cfg.NF, cfg.NTF
    NTp, C = struct["NTp"], struct["C"]
    W, colstart, chunks = struct["W"], struct["colstart"], struct["chunks"]

    nc = bacc.Bacc(
        "TRN2", target_bir_lowering=False, debug=False, num_devices=NC
    )

    # I/O
    hTb = nc.dram_tensor("hTb", [P, KCH * NF], BF16, kind="ExternalInput").ap()
    hTownb = nc.dram_tensor("hTownb", [P, KCH * NPAD], BF16, kind="ExternalInput").ap()
    W_augb = nc.dram_tensor("W_augb", [P, KCH * 130], BF16, kind="ExternalInput").ap()
    asrcb = nc.dram_tensor("asrcb", [P, 128], BF16, kind="ExternalInput").ap()
    ident_in = nc.dram_tensor("ident", [P, P], BF16, kind="ExternalInput").ap()
    gidx_in = nc.dram_tensor("gidx", [2, P, C * 8], I16, kind="ExternalInput").ap()
    gmask_in = nc.dram_tensor("gmask", [2, P, C], F32, kind="ExternalInput").ap()
    mscat_in = nc.dram_tensor("mscat", [2, P, NTp * 8], I16, kind="ExternalInput").ap()
    zdscat_in = nc.dram_tensor("zdscat", [2, P, NPAD // 16], I16, kind="ExternalInput").ap()

    out = nc.dram_tensor("out", [NPAD, OUT_DIM], F32, kind="ExternalOutput").ap()
    acc = nc.dram_tensor("acc", [NPAD, ACC2], F32, kind="ExternalOutput").ap()
    zdperm = nc.dram_tensor("zdperm", [2 * NPAD, ZDP_STRIDE], F32, kind="ExternalOutput").ap()

    # two half tables so half-0 gathers need not wait for half-1 writes
    tables = [
        nc.dram_tensor(f"table{h}", [cfg.HROWS[h], ROW], BF16, kind="Internal").ap()
        for h in (0, 1)
    ]

    nc.gpsimd.load_library(library_config.mlp)

    with tile.TileContext(nc) as tc:
        with tc.tile_pool(name="const", bufs=1) as constp:
            ident = constp.tile([P, P], BF16)
            nc.scalar.dma_start(ident, ident_in)
            asrc = constp.tile([P, 128], BF16)
            nc.scalar.dma_start(asrc, asrcb)
            wsb = constp.tile([P, KCH, 130], BF16)
            nc.scalar.dma_start(wsb, W_augb.rearrange("p (ko m) -> p ko m", ko=KCH))

            # edge metadata early so phase-C gathers aren't gated on DMA order
            gidx_sb = []
            gmask_sb = []
            mscat_sb = []
            for h in (0, 1) if stage >= 4 else ():
                g = constp.tile([P, C * 8], I16, tag=f"gidx{h}")
                nc.scalar.dma_start(g, gidx_in[h])
                gidx_sb.append(g)
                m = constp.tile([P, C], F32, tag=f"gmask{h}")
                nc.scalar.dma_start(m, gmask_in[h])
                gmask_sb.append(m)
                s = constp.tile([P, NTp * 8], I16, tag=f"mscat{h}")
                nc.scalar.dma_start(s, mscat_in[h])
                mscat_sb.append(s)

            # phase-C pools open FIRST so their SBUF zones are disjoint from
            # phase-B's (address reuse would turn gathers' G-tile writes into
            # false WAR deps on phase-B's last matmul).  Emission interleaves
            # the phases: B chunks covering table0 -> C half 0 -> remaining B
            # chunks -> C half 1, so half-0 edge work overlaps half-1 z work.
            with (
                tc.tile_pool(name="meta", bufs=1) as metap,
                tc.tile_pool(name="gbuf", bufs=3) as gbuf,
                tc.tile_pool(name="ebig", bufs=3) as ebig,
                tc.tile_pool(name="ebuf", bufs=2) as ebuf,
                tc.tile_pool(name="aggb", bufs=1) as aggb,
                tc.tile_pool(name="ps2", bufs=2, space="PSUM") as ps2,
                tc.tile_pool(name="ph1h", bufs=2) as ph1h,
                tc.tile_pool(name="ph1r", bufs=2) as ph1r,
                tc.tile_pool(name="ph1ps", bufs=2, space="PSUM") as ph1ps,
            ):

                def emit_b_chunk(c0):
                    csz = min(HCHUNK, NF - c0)
                    hsb = ph1h.tile([P, KCH, HCHUNK], BF16, tag="hsb")
                    nc.sync.dma_start(
                        hsb[:, :, 0:csz],
                        hTb.rearrange("p (ko n) -> p ko n", ko=KCH)[:, :, c0:c0 + csz],
                    )
                    rows = ph1r.tile([P, HCHUNK // P, ROW], BF16, tag="rows")
                    for g0 in range(0, csz, GRP * P):
                        gsz = min(GRP * P, csz - g0)
                        ntg = gsz // P
                        ps3 = ph1ps.tile([P, GRP, 512], F32)
                        for i in range(ntg):
                            for k in range(KCH):
                                nc.tensor.matmul(
                                    ps3[:, i, 0:128],
                                    lhsT=hsb[:, k, g0 + i * P : g0 + (i + 1) * P],
                                    rhs=wsb[:, k, 0:128],
                                    start=(k == 0),
                                    stop=(k == KCH - 1),
                                )
                        t0 = g0 // P
                        nc.scalar.copy(rows[:, t0 : t0 + ntg, :], ps3[:, 0:ntg, 0:128])
                    # one batched table write per chunk, split at the half
                    # boundary so each half-table's writes finish ASAP
                    spans = []
                    if c0 < cfg.HALF:
                        hi = min(c0 + csz, cfg.HALF)
                        spans.append((0, c0, hi - c0))
                    if c0 + csz > cfg.HALF:
                        lo = max(c0, cfg.HALF)
                        spans.append((1, lo - cfg.HALF, c0 + csz - lo))
                    for (hh, r0, rn) in spans:
                        toff = (cfg.HB[hh] + r0) - c0  # offset in rows tile
                        nc.sync.dma_start(
                            tables[hh][r0 : r0 + rn, :].rearrange(
                                "(t p) d -> p t d", p=P
                            ),
                            rows[:, toff // P : (toff + rn) // P, :],
                        )

                def emit_mini():
                    """Own-node zd (cols of the augmented matmul) + slot
                    permute.  Runs between table0 chunks and half-0 edge work
                    so its matmuls don't block phase-B's PE queue."""
                    assert HCHUNK == NT * P == NPAD
                    hto = ph1h.tile([P, KCH, HCHUNK], BF16, tag="hsb")
                    nc.scalar.dma_start(
                        hto,
                        hTownb.rearrange("p (ko n) -> p ko n", ko=KCH),
                    )
                    psz = ps2.tile([P, SCAT_ELEMS], F32, tag="aggps")
                    for t in range(NT):
                        for k in range(KCH):
                            nc.tensor.matmul(
                                psz[:, t : t + 1],
                                lhsT=hto[:, k, ts(t, P)],
                                rhs=wsb[:, k, 129:130],
                                start=(k == 0),
                                stop=(k == KCH - 1),
                            )
                    zd_sb = metap.tile([P, NT, 1], F32, tag="zdsb")
                    nc.vector.tensor_copy(zd_sb[:, :, 0], psz[:, 0:NT])
                    for h in (0, 1) if stage >= 2 else ():
                        zdsc = metap.tile([P, NPAD // 16], I16, tag=f"zdsc{h}")
                        nc.scalar.dma_start(zdsc, zdscat_in[h])
                        nc.gpsimd.dma_scatter_add(
                            out_ap=zdperm[h * NPAD:(h + 1) * NPAD, 0:1],
                            in_ap=zd_sb,
                            idxs_ap=zdsc,
                            num_idxs=NPAD,
                            num_idxs_reg=NPAD,
                            elem_size=1,
                            elem_step=ZDP_STRIDE,
                            single_packet=NPAD <= 1024,
                        )

                def emit_c_half(h):
                    z = metap.tile([P, NT, 1], F32, tag=f"zds{h}")
                    nc.scalar.dma_start(
                        z,
                        zdperm[h * NPAD:(h + 1) * NPAD, :]
                        .rearrange("(t p) d -> p t d", p=P)[:, :, 0:1],
                    )
                    if stage < 5:
                        return
                    agg = aggb.tile([P, NTp, SCAT_ELEMS], F32, tag=f"agg{h}")
                    for (t0, ntc, w) in chunks:
                        cc = ntc * w
                        c0 = int(colstart[t0])
                        G = gbuf.tile([P, CHUNK_COLS, ROW], BF16, tag="G")
                        nc.gpsimd.dma_gather(
                            out_ap=G[:, 0:cc, :],
                            in_ap=tables[h],
                            idxs_ap=gidx_sb[h][:, c0 * 8:(c0 + cc) * 8],
                            num_idxs=cc * P,
                            num_idxs_reg=cc * P,
                            elem_size=ROW,
                            single_packet=cc * P <= 1024,
                        )
                        if sub < 2:
                            continue
                        # zs[src] = gathered z . a_src (free-axis dot)
                        zsp = ebig.tile([P, CHUNK_COLS, ROW], BF16, tag="zsp")
                        nc.vector.tensor_tensor(
                            zsp[:, 0:cc, :],
                            G[:, 0:cc, :],
                            asrc[:, None, :].to_broadcast([P, cc, ROW]),
                            mybir.AluOpType.mult,
                        )
                        zsc = ebuf.tile([P, CHUNK_COLS], F32, tag="zsc")
                        nc.vector.tensor_reduce(
                            zsc[:, 0:cc],
                            zsp[:, 0:cc, :],
                            mybir.AxisListType.X,
                            mybir.AluOpType.add,
                        )
                        score = ebuf.tile([P, CHUNK_COLS], F32, tag="score")
                        sc = score[:, 0:cc].rearrange("p (t w) -> p t w", w=w)
                        nc.vector.tensor_tensor(
                            sc,
                            zsc[:, 0:cc].rearrange("p (t w) -> p t w", w=w),
                            z[:, t0:t0 + ntc, :].to_broadcast([P, ntc, w]),
                            mybir.AluOpType.add,
                        )
                        exf = ebuf.tile([P, CHUNK_COLS], F32, tag="exf")
                        nc.vector.scalar_tensor_tensor(
                            exf[:, 0:cc], score[:, 0:cc], 0.01, score[:, 0:cc],
                            op0=mybir.AluOpType.mult, op1=mybir.AluOpType.max,
                        )
                        nc.scalar.activation(
                            exf[:, 0:cc], exf[:, 0:cc],
                            mybir.ActivationFunctionType.Exp,
                        )
                        exm = ebuf.tile([P, CHUNK_COLS], BF16, tag="exm")
                        nc.vector.tensor_tensor(
                            exm[:, 0:cc], exf[:, 0:cc],
                            gmask_sb[h][:, c0:c0 + cc],
                            mybir.AluOpType.mult,
                        )
                        # exz rows [z*exm | exm]: col 128 makes the PE
                        # chain accumulate the denominator for free
                        exz = ebig.tile([P, CHUNK_COLS, SCAT_ELEMS], BF16, tag="exz")
                        nc.vector.tensor_tensor(
                            exz[:, 0:cc, 0:ROW],
                            G[:, 0:cc, :],
                            exm[:, 0:cc, None].to_broadcast([P, cc, ROW]),
                            mybir.AluOpType.mult,
                        )
                        nc.vector.tensor_copy(exz[:, 0:cc, ROW], exm[:, 0:cc])
                        if sub < 3:
                            continue
                        for ti in range(ntc):
                            ps = ps2.tile([P, SCAT_ELEMS], F32, tag="aggps")
                            for r in range(w):
                                nc.tensor.matmul(
                                    ps,
                                    lhsT=ident,
                                    rhs=exz[:, ti * w + r, :],
                                    start=(r == 0),
                                    stop=(r == w - 1),
                                )
                            nc.scalar.copy(agg[:, t0 + ti, 0:SCAT_ELEMS], ps)
                    if sub < 4:
                        return
                    nc.gpsimd.dma_scatter_add(
                        out_ap=acc[:, 0:SCAT_ELEMS],
                        in_ap=agg,
                        idxs_ap=mscat_sb[h],
                        num_idxs=NTp * P,
                        num_idxs_reg=NTp * P,
                        elem_size=SCAT_ELEMS,
                        elem_step=ACC2,
                        single_packet=NTp * P <= 1024,
                    )

                b_chunks = list(range(0, NF, HCHUNK)) if stage >= 3 else []
                b_first = [c for c in b_chunks if c < cfg.HALF]
                b_rest = [c for c in b_chunks if c >= cfg.HALF]
                for c0 in b_first:
                    emit_b_chunk(c0)
                emit_mini()
                if stage >= 4:
                    emit_c_half(0)
                for c0 in b_rest:
                    emit_b_chunk(c0)
                if stage >= 4:
                    emit_c_half(1)

            # ---------------- phase D: divide + output ----------------
            with tc.tile_pool(name="fin", bufs=1) as finp:
                accs = finp.tile([P, NT, SCAT_ELEMS], F32)
                av = acc.rearrange("(t p) d -> p t d", p=P)
                nc.sync.dma_start(accs, av[:, :, 0:SCAT_ELEMS])
                den = finp.tile([P, NT], F32)
                nc.vector.tensor_copy(den, accs[:, :, 128])
                nc.vector.tensor_scalar(
                    den, den, 1e-30, None, mybir.AluOpType.max
                )
                rec = finp.tile([P, NT], F32)
                nc.vector.reciprocal(rec, den)
                res = finp.tile([P, NT, OUT_DIM], F32)
                nc.vector.tensor_tensor(
                    res,
                    accs[:, :, 0:OUT_DIM],
                    rec[:, :, None].to_broadcast([P, NT, OUT_DIM]),
                    mybir.AluOpType.mult,
                )
                nc.scalar.dma_start(out.rearrange("(t p) d -> p t d", p=P), res)

    nc.finalize()
    return nc


def _pack_kmajor(mat, ncols):
    """[256, ncols] f32 -> [128, 2*ncols] bf16 with row (ko*128+ki) at
    [ki, ko*ncols : (ko+1)*ncols] (one contiguous run per partition)."""
    b = mat.astype(ml_dtypes.bfloat16)
    return np.ascontiguousarray(
        b.reshape(KCH, P, ncols).transpose(1, 0, 2).reshape(P, KCH * ncols)
    )


def make_in_maps(cfg, struct, data, h, W_fc, a_attn):
    NPC, NPAD, NF = cfg.NPC, cfg.NPAD, cfg.NF
    h = np.asarray(h, np.float32)
    W_fc = np.asarray(W_fc, np.float32)
    a_attn = np.asarray(a_attn, np.float32)

    w_s = W_fc.T @ a_attn[:OUT_DIM]     # [256]
    w_d = W_fc.T @ a_attn[OUT_DIM:]
    W_aug = np.concatenate(
        [W_fc.T, w_s[:, None], w_d[:, None]], axis=1
    ).astype(np.float32)                # [256, 130]
    W_augb = _pack_kmajor(W_aug, 130)

    hT_full = np.zeros((IN_DIM, NF), np.float32)
    hT_full[:, : cfg.N] = h.T
    hTb = _pack_kmajor(hT_full, NF)

    asrc_b = np.ascontiguousarray(
        np.tile(a_attn[:OUT_DIM].astype(ml_dtypes.bfloat16)[None, :], (P, 1))
    )
    ident = np.eye(P, dtype=ml_dtypes.bfloat16)

    in_maps = []
    for c in range(NC):
        hTo = np.zeros((IN_DIM, NPAD), np.float32)
        hTo[:, :NPC] = hT_full[:, c * NPC:(c + 1) * NPC]
        gidx = np.stack([data[(c, 0)]["gidx"], data[(c, 1)]["gidx"]])
        gmask = np.stack([data[(c, 0)]["gmask"], data[(c, 1)]["gmask"]])
        mscat = np.stack([data[(c, 0)]["mscat"], data[(c, 1)]["mscat"]])
        zdscat = np.stack([data[(c, 0)]["zdscat"], data[(c, 1)]["zdscat"]])
        in_maps.append({
            "hTb": hTb,
            "hTownb": _pack_kmajor(hTo, NPAD),
            "W_augb": W_augb,
            "asrcb": asrc_b,
            "ident": ident,
            "gidx": np.ascontiguousarray(gidx),
            "gmask": np.ascontiguousarray(gmask),
            "mscat": np.ascontiguousarray(mscat),
            "zdscat": np.ascontiguousarray(zdscat),
        })
    return in_maps


def run(h, src, dst, W_fc, a_attn, n_nodes=None, n_edges=None, trace=False):
    h = np.asarray(h, np.float32)
    cfg = Cfg(
        n_nodes if n_nodes is not None else h.shape[0],
        n_edges if n_edges is not None else np.asarray(src).shape[0],
    )
    struct, data = host_prep(cfg, src, dst)
    nc = build_program(cfg, struct)
    in_maps = make_in_maps(cfg, struct, data, h, W_fc, a_attn)
    results = run_bass_kernel_spmd(
        nc, in_maps, core_ids=list(range(NC)), trace=trace
    )
    outs = [r["out"] for r in results.results]
    full = np.concatenate([o[: cfg.NPC] for o in outs], axis=0).astype(np.float32)
    return full, results


def kernel(h, src, dst, W_fc, a_attn):
    full, _ = run(h, src, dst, W_fc, a_attn)
    return full


# revision 34
# speedup vs baseline: 1.3586x; 1.3586x over previous
"""GAT layer (gnn_message_passing) on 8 Trainium2 NeuronCores.

v2 strategy (replicated z, zero collectives, dst-owner edge sharding):
  - Every core computes the FULL z = h @ W.T table (h shipped bf16,
    ~25.6MB load + ~45us of bf16 matmul) and writes it to a local DRAM
    table of pure-z 256B rows (identity node->row mapping).  This replaces
    the v1 sharded-z + AllGather (283us of collective on the sim critical
    path) and removes every inter-core sync point.
  - Edges are bucketed by the owner of dst (6250 nodes/core), so segment
    softmax is fully local.  Softmax uses the no-max-subtraction form.
  - Per (core, half) the owned nodes are degree-sorted into a dst-major
    slot layout: partition p = node slot, free column = edge rank.
    dma_gather (int16 indices -> table split in two halves at node 25088)
    fetches z[src] rows straight into that layout.
  - zs[src] per edge is recomputed on the vector engine from the gathered
    bf16 z rows (dot with a_src along the free axis) instead of being
    stored in the table -- keeps gather rows at 256B (vs 512B in v1).
  - zd for owned nodes comes from a tiny per-core matmul over the core's
    own h slice (rhs = w_d column only), permuted to slot order via a
    dma_scatter_add round trip through DRAM (zdperm).
  - Weighted segment-sum runs on the TensorEngine as identity-lhsT matmul
    chains accumulating in PSUM; denominators via a vector reduce.
  - Both halves scatter-add into the SAME 129-col accumulator block
    (scatter_add merges them), then a final divide produces out.
  - Zero-in-degree nodes get a fake self-edge on the host so out == z.
"""

import numpy as np
import ml_dtypes

import concourse.bass as bass
import concourse.mybir as mybir
import concourse.tile as tile
from concourse import bacc
from concourse import library_config
from concourse.bass import ts
from concourse.bass_utils import run_bass_kernel_spmd

F32 = mybir.dt.float32
BF16 = mybir.dt.bfloat16
I16 = mybir.dt.int16

NC = 8          # cores
P = 128         # partitions
IN_DIM = 256
OUT_DIM = 128
KCH = IN_DIM // P       # 2 k-chunks for the feature matmuls
ROW = 128               # bf16 elems per table row (256 B, pure z)
ACC2 = 256              # bf16 elems per accumulator row (512 B)
GCOLS = 64              # gather columns per dma_gather call (2 chunks)
SCAT_ELEMS = 129        # f32 elems scattered per slot ([agg128 | den])
ZDP_STRIDE = 64         # f32 stride of the zd permute buffer (256 B)
CHUNK_COLS = 32         # max gather columns per chunk
HCHUNK = 6272           # h columns per phase-1 SBUF chunk (49 tiles; 4 chunks = table0)
GRP = 3                 # tiles per phase-1 PSUM group (3 PSUM banks)


class Cfg:
    def __init__(self, n_nodes, n_edges):
        assert n_nodes % NC == 0
        self.N = n_nodes
        self.E = n_edges
        self.NPC = n_nodes // NC                  # 6250 owned nodes/core
        self.NPAD = ((self.NPC + P - 1) // P) * P  # 6400
        self.NT = self.NPAD // P                   # 50 owned tiles
        self.NF = ((n_nodes + P - 1) // P) * P     # 50048 full-table rows
        self.NTF = self.NF // P                    # 391 full tiles
        # table halves split at a tile boundary so int16 gather indices fit
        self.HALF = (self.NTF // 2 + 1) * P        # 25088
        self.HB = [0, self.HALF]
        self.HROWS = [self.HALF, self.NF - self.HALF]
        assert max(self.HROWS) < 32768


def _wrap16(flat, dtype=np.int16):
    """flat[i] -> [128, len/16] with flat[i] at [i%16, i//16], replicated x8."""
    n = flat.shape[0]
    assert n % 16 == 0
    w = flat.reshape(n // 16, 16).T.astype(dtype)  # [16, n/16]
    return np.tile(w, (8, 1))


def host_prep(cfg, src, dst):
    """Build the common tile structure + per-core index/mask arrays."""
    N, NPC, NPAD, HALF = cfg.N, cfg.NPC, cfg.NPAD, cfg.HALF
    src = np.asarray(src, np.int64).copy()
    dst = np.asarray(dst, np.int64).copy()

    # fake self-edges for isolated (zero in-degree) nodes -> out == z exactly
    deg_tot = np.bincount(dst, minlength=N)
    iso = np.nonzero(deg_tot == 0)[0]
    if iso.size:
        src = np.concatenate([src, iso])
        dst = np.concatenate([dst, iso])

    owner = dst // NPC
    halves = (src >= HALF).astype(np.int64)
    table_local = src - halves * HALF            # identity node layout
    assert table_local.max() < 32768 and table_local.min() >= 0

    per = {}
    deg_sorted_all = []
    for c in range(NC):
        for h in (0, 1):
            m = (owner == c) & (halves == h)
            es = table_local[m]               # gather index of each edge
            ed = dst[m] - c * NPC             # local dst node
            deg = np.bincount(ed, minlength=NPAD)
            order = np.argsort(-deg, kind="stable")  # node_of_slot [NPAD]
            sL = np.empty(NPAD, np.int64)
            sL[order] = np.arange(NPAD)              # slot of node
            per[(c, h)] = dict(es=es, ed=ed, deg=deg, order=order, sL=sL)
            deg_sorted_all.append(deg[order])

    # common tile widths
    NT = cfg.NT
    W = np.zeros(NT, np.int64)
    for dsrt in deg_sorted_all:
        W = np.maximum(W, dsrt[::P][:NT])
    NTp = int(np.nonzero(W > 0)[0][-1]) + 1 if (W > 0).any() else 0
    W = W[:NTp]
    colstart = np.concatenate([[0], np.cumsum(W)]).astype(np.int64)
    C = int(colstart[-1])
    CP = C  # gather cols total per half

    # chunks: runs of equal W, capped at CHUNK_COLS columns
    chunks = []  # (t0, nt, W)
    t = 0
    while t < NTp:
        w = int(W[t])
        nt = 1
        while (
            t + nt < NTp
            and int(W[t + nt]) == w
            and (nt + 1) * w <= CHUNK_COLS
        ):
            nt += 1
        chunks.append((t, nt, w))
        t += nt

    # per-(core,half) flat arrays
    data = {}
    for c in range(NC):
        for h in (0, 1):
            d = per[(c, h)]
            es, ed, deg, order, sL = (
                d["es"], d["ed"], d["deg"], d["order"], d["sL"],
            )
            slot = sL[ed]
            tile_of = slot // P
            part_of = slot % P
            # rank of each edge within its dst node
            o = np.argsort(slot, kind="stable")
            slot_s = slot[o]
            es_s = es[o]
            tile_s = tile_of[o]
            part_s = part_of[o]
            counts = np.bincount(slot_s, minlength=NPAD)
            starts = np.concatenate([[0], np.cumsum(counts)])[:-1]
            rank = np.arange(slot_s.size) - starts[slot_s]
            keep = tile_s < NTp
            assert keep.all(), "edge landed outside processed tiles"
            cglob = colstart[tile_s] + rank
            assert (rank < W[tile_s]).all()
            pos = cglob * P + part_s

            flat_idx = np.zeros(CP * P, np.int16)
            flat_idx[pos] = es_s.astype(np.int16)
            # additive mask: 0 at real edges, -1e30 at padding (exp -> 0)
            mask = np.full((P, CP), -1e30, np.float32)
            mask[part_s, cglob] = 0.0

            data[(c, h)] = dict(
                gidx=_wrap16(flat_idx),
                gmask=mask,
                mscat=_wrap16(order[: NTp * P].astype(np.int16)),
                zdscat=_wrap16(sL.astype(np.int16)),
            )

    # pack consecutive chunks into gather ranges of <= GCOLS columns
    ranges = []  # (col0, ccr, [(t0, ntc, w, off), ...])
    cur = None
    for (t0, nt_, w) in chunks:
        cc = nt_ * w
        col0 = int(colstart[t0])
        if cur is not None and (col0 + cc - cur[0]) <= GCOLS:
            cur[2].append((t0, nt_, w, col0 - cur[0]))
            cur[1] = col0 + cc - cur[0]
        else:
            cur = [col0, cc, [(t0, nt_, w, 0)]]
            ranges.append(cur)
    ranges = [(r[0], r[1], r[2]) for r in ranges]

    struct = dict(W=W, NTp=NTp, colstart=colstart, C=C, chunks=chunks,
                  ranges=ranges)
    return struct, data


def build_program(cfg, struct, stage=5, sub=4):
    NPAD, NT, NF, NTF = cfg.NPAD, cfg.NT, cfg.NF, cfg.NTF
    NTp, C = struct["NTp"], struct["C"]
    W, colstart, chunks = struct["W"], struct["colstart"], struct["chunks"]
    ranges = struct["ranges"]

    nc = bacc.Bacc(
        "TRN2", target_bir_lowering=False, debug=False, num_devices=NC
    )

    # I/O
    hTb = nc.dram_tensor("hTb", [P, KCH * NF], BF16, kind="ExternalInput").ap()
    hTownb = nc.dram_tensor("hTownb", [P, KCH * NPAD], BF16, kind="ExternalInput").ap()
    W_augb = nc.dram_tensor("W_augb", [P, KCH * 130], BF16, kind="ExternalInput").ap()
    asrcb = nc.dram_tensor("asrcb", [P, 128], BF16, kind="ExternalInput").ap()
    ident_in = nc.dram_tensor("ident", [P, P], BF16, kind="ExternalInput").ap()
    gidx_in = nc.dram_tensor("gidx", [2, P, C * 8], I16, kind="ExternalInput").ap()
    gmask_in = nc.dram_tensor("gmask", [2, P, C], F32, kind="ExternalInput").ap()
    mscat_in = nc.dram_tensor("mscat", [2, P, NTp * 8], I16, kind="ExternalInput").ap()
    zdscat_in = nc.dram_tensor("zdscat", [2, P, NPAD // 16], I16, kind="ExternalInput").ap()

    out = nc.dram_tensor("out", [NPAD, OUT_DIM], F32, kind="ExternalOutput").ap()
    acc = nc.dram_tensor("acc", [NPAD, ACC2], BF16, kind="ExternalOutput").ap()
    zdperm = nc.dram_tensor("zdperm", [2 * NPAD, ZDP_STRIDE], F32, kind="ExternalOutput").ap()

    # two half tables so half-0 gathers need not wait for half-1 writes
    tables = [
        nc.dram_tensor(f"table{h}", [cfg.HROWS[h], ROW], BF16, kind="Internal").ap()
        for h in (0, 1)
    ]

    nc.gpsimd.load_library(library_config.mlp)

    with tile.TileContext(nc) as tc:
        with tc.tile_pool(name="const", bufs=1) as constp:
            ident = constp.tile([P, P], BF16)
            nc.scalar.dma_start(ident, ident_in)
            asrc = constp.tile([P, 128], BF16)
            nc.scalar.dma_start(asrc, asrcb)
            wsb = constp.tile([P, KCH, 130], BF16)
            nc.scalar.dma_start(wsb, W_augb.rearrange("p (ko m) -> p ko m", ko=KCH))

            # edge metadata early so phase-C gathers aren't gated on DMA order
            gidx_sb = []
            gmask_sb = []
            mscat_sb = []
            for h in (0, 1) if stage >= 4 else ():
                g = constp.tile([P, C * 8], I16, tag=f"gidx{h}")
                nc.scalar.dma_start(g, gidx_in[h])
                gidx_sb.append(g)
                m = constp.tile([P, C], F32, tag=f"gmask{h}")
                nc.scalar.dma_start(m, gmask_in[h])
                gmask_sb.append(m)
                s = constp.tile([P, NTp * 8], I16, tag=f"mscat{h}")
                nc.scalar.dma_start(s, mscat_in[h])
                mscat_sb.append(s)

            # phase-C pools open FIRST so their SBUF zones are disjoint from
            # phase-B's (address reuse would turn gathers' G-tile writes into
            # false WAR deps on phase-B's last matmul).  Emission interleaves
            # the phases: B chunks covering table0 -> C half 0 -> remaining B
            # chunks -> C half 1, so half-0 edge work overlaps half-1 z work.
            with (
                tc.tile_pool(name="meta", bufs=1) as metap,
                tc.tile_pool(name="gbuf", bufs=2) as gbuf,
                tc.tile_pool(name="ebig", bufs=3) as ebig,
                tc.tile_pool(name="ebuf", bufs=2) as ebuf,
                tc.tile_pool(name="aggb", bufs=1) as aggb,
                tc.tile_pool(name="ps2", bufs=2, space="PSUM") as ps2,
                tc.tile_pool(name="ph1h", bufs=2) as ph1h,
                tc.tile_pool(name="ph1r", bufs=2) as ph1r,
                tc.tile_pool(name="ph1ps", bufs=2, space="PSUM") as ph1ps,
            ):

                def emit_b_chunk(c0):
                    csz = min(HCHUNK, NF - c0)
                    hsb = ph1h.tile([P, KCH, HCHUNK], BF16, tag="hsb")
                    nc.sync.dma_start(
                        hsb[:, :, 0:csz],
                        hTb.rearrange("p (ko n) -> p ko n", ko=KCH)[:, :, c0:c0 + csz],
                    )
                    rows = ph1r.tile([P, HCHUNK // P, ROW], BF16, tag="rows")
                    for gi, g0 in enumerate(range(0, csz, GRP * P)):
                        gsz = min(GRP * P, csz - g0)
                        ntg = gsz // P
                        ps3 = ph1ps.tile([P, GRP, 512], F32)
                        for i in range(ntg):
                            for k in range(KCH):
                                nc.tensor.matmul(
                                    ps3[:, i, 0:128],
                                    lhsT=hsb[:, k, g0 + i * P : g0 + (i + 1) * P],
                                    rhs=wsb[:, k, 0:128],
                                    start=(k == 0),
                                    stop=(k == KCH - 1),
                                )
                        t0 = g0 // P
                        nc.scalar.copy(rows[:, t0 : t0 + ntg, :], ps3[:, 0:ntg, 0:128])
                    # one batched table write per chunk, split at the half
                    # boundary so each half-table's writes finish ASAP
                    spans = []
                    if c0 < cfg.HALF:
                        hi = min(c0 + csz, cfg.HALF)
                        spans.append((0, c0, hi - c0))
                    if c0 + csz > cfg.HALF:
                        lo = max(c0, cfg.HALF)
                        spans.append((1, lo - cfg.HALF, c0 + csz - lo))
                    for (hh, r0, rn) in spans:
                        toff = (cfg.HB[hh] + r0) - c0  # offset in rows tile
                        nc.sync.dma_start(
                            tables[hh][r0 : r0 + rn, :].rearrange(
                                "(t p) d -> p t d", p=P
                            ),
                            rows[:, toff // P : (toff + rn) // P, :],
                        )

                def emit_mini():
                    """Own-node zd (cols of the augmented matmul) + slot
                    permute.  Runs between table0 chunks and half-0 edge work
                    so its matmuls don't block phase-B's PE queue."""
                    assert HCHUNK == NT * P == NPAD
                    hto = ph1h.tile([P, KCH, HCHUNK], BF16, tag="hsb")
                    nc.gpsimd.dma_start(
                        hto,
                        hTownb.rearrange("p (ko n) -> p ko n", ko=KCH),
                    )
                    psz = ps2.tile([P, SCAT_ELEMS], F32, tag="aggps")
                    for t in range(NT):
                        for k in range(KCH):
                            nc.tensor.matmul(
                                psz[:, t : t + 1],
                                lhsT=hto[:, k, ts(t, P)],
                                rhs=wsb[:, k, 129:130],
                                start=(k == 0),
                                stop=(k == KCH - 1),
                            )
                    zd_sb = metap.tile([P, NT, 1], F32, tag="zdsb")
                    nc.vector.tensor_copy(zd_sb[:, :, 0], psz[:, 0:NT])
                    for h in (0, 1) if stage >= 2 else ():
                        zdsc = metap.tile([P, NPAD // 16], I16, tag=f"zdsc{h}")
                        nc.scalar.dma_start(zdsc, zdscat_in[h])
                        nc.gpsimd.dma_scatter_add(
                            out_ap=zdperm[h * NPAD:(h + 1) * NPAD, 0:1],
                            in_ap=zd_sb,
                            idxs_ap=zdsc,
                            num_idxs=NPAD,
                            num_idxs_reg=NPAD,
                            elem_size=1,
                            elem_step=ZDP_STRIDE,
                            single_packet=NPAD <= 1024,
                        )

                def emit_c_half(h):
                    z = metap.tile([P, NT, 1], F32, tag=f"zds{h}")
                    nc.scalar.dma_start(
                        z,
                        zdperm[h * NPAD:(h + 1) * NPAD, :]
                        .rearrange("(t p) d -> p t d", p=P)[:, :, 0:1],
                    )
                    if stage < 5:
                        return
                    agg = aggb.tile([P, NTp, SCAT_ELEMS], BF16, tag=f"agg{h}")
                    for (col0, ccr, rchunks) in ranges:
                        G = gbuf.tile([P, GCOLS, ROW], BF16, tag="G")
                        nc.gpsimd.dma_gather(
                            out_ap=G[:, 0:ccr, :],
                            in_ap=tables[h],
                            idxs_ap=gidx_sb[h][:, col0 * 8:(col0 + ccr) * 8],
                            num_idxs=ccr * P,
                            num_idxs_reg=ccr * P,
                            elem_size=ROW,
                            single_packet=ccr * P <= 1024,
                        )
                        if sub < 2:
                            continue
                        # zs[src] = gathered z . a_src (free-axis dot),
                        # range-wide ops amortize per-op overheads
                        zsp = ebig.tile([P, GCOLS, SCAT_ELEMS], BF16, tag="exz")
                        nc.vector.tensor_tensor(
                            zsp[:, 0:ccr, 0:ROW],
                            G[:, 0:ccr, :],
                            asrc[:, None, :].to_broadcast([P, ccr, ROW]),
                            mybir.AluOpType.mult,
                        )
                        zsc = ebuf.tile([P, GCOLS], F32, tag="zsc")
                        nc.vector.tensor_reduce(
                            zsc[:, 0:ccr],
                            zsp[:, 0:ccr, 0:ROW],
                            mybir.AxisListType.X,
                            mybir.AluOpType.add,
                        )
                        # scores: zsc + additive pad mask, + zd (per chunk),
                        # then range-wide leaky + exp
                        score = ebuf.tile([P, GCOLS], F32, tag="score")
                        nc.vector.tensor_tensor(
                            score[:, 0:ccr], zsc[:, 0:ccr],
                            gmask_sb[h][:, col0:col0 + ccr],
                            mybir.AluOpType.add,
                        )
                        for (t0, ntc, w, off) in rchunks:
                            sc = score[:, off:off + ntc * w].rearrange(
                                "p (t w) -> p t w", w=w
                            )
                            nc.vector.tensor_tensor(
                                sc,
                                sc,
                                z[:, t0:t0 + ntc, :].to_broadcast([P, ntc, w]),
                                mybir.AluOpType.add,
                            )
                        exf = ebuf.tile([P, GCOLS], F32, tag="exf")
                        nc.vector.scalar_tensor_tensor(
                            exf[:, 0:ccr], score[:, 0:ccr], 0.01, score[:, 0:ccr],
                            op0=mybir.AluOpType.mult, op1=mybir.AluOpType.max,
                        )
                        exm = ebuf.tile([P, GCOLS], BF16, tag="exm")
                        nc.scalar.activation(
                            exm[:, 0:ccr], exf[:, 0:ccr],
                            mybir.ActivationFunctionType.Exp,
                        )
                        # exz rows [z*exm | exm]: col 128 makes the PE
                        # chain accumulate the denominator for free
                        exz = ebig.tile([P, GCOLS, SCAT_ELEMS], BF16, tag="exz")
                        nc.vector.tensor_tensor(
                            exz[:, 0:ccr, 0:ROW],
                            G[:, 0:ccr, :],
                            exm[:, 0:ccr, None].to_broadcast([P, ccr, ROW]),
                            mybir.AluOpType.mult,
                        )
                        nc.vector.tensor_copy(exz[:, 0:ccr, ROW], exm[:, 0:ccr])
                        if sub < 3:
                            continue
                        for (t0, ntc, w, off) in rchunks:
                            for ti in range(ntc):
                                ps = ps2.tile([P, SCAT_ELEMS], F32, tag="aggps")
                                for r in range(w):
                                    nc.tensor.matmul(
                                        ps,
                                        lhsT=ident,
                                        rhs=exz[:, off + ti * w + r, :],
                                        start=(r == 0),
                                        stop=(r == w - 1),
                                    )
                                nc.scalar.copy(agg[:, t0 + ti, 0:SCAT_ELEMS], ps)
                    if sub < 4:
                        return
                    nc.gpsimd.dma_scatter_add(
                        out_ap=acc[:, 0:SCAT_ELEMS],
                        in_ap=agg,
                        idxs_ap=mscat_sb[h],
                        num_idxs=NTp * P,
                        num_idxs_reg=NTp * P,
                        elem_size=SCAT_ELEMS,
                        elem_step=ACC2,
                        single_packet=NTp * P <= 1024,
                    )

                b_chunks = list(range(0, NF, HCHUNK)) if stage >= 3 else []
                b_first = [c for c in b_chunks if c < cfg.HALF]
                b_rest = [c for c in b_chunks if c >= cfg.HALF]
                for ci, c0 in enumerate(b_first):
                    emit_b_chunk(c0)
                    if ci == 0:
                        emit_mini()
                if stage >= 4:
                    emit_c_half(0)
                for c0 in b_rest:
                    emit_b_chunk(c0)
                if stage >= 4:
                    emit_c_half(1)

            # ---------------- phase D: divide + output ----------------
            with tc.tile_pool(name="fin", bufs=1) as finp:
                accs = finp.tile([P, NT, SCAT_ELEMS], BF16)
                av = acc.rearrange("(t p) d -> p t d", p=P)
                nc.sync.dma_start(accs, av[:, :, 0:SCAT_ELEMS])
                den = finp.tile([P, NT], F32)
                nc.vector.tensor_copy(den, accs[:, :, 128])
                nc.vector.tensor_scalar(
                    den, den, 1e-30, None, mybir.AluOpType.max
                )
                rec = finp.tile([P, NT], F32)
                nc.vector.reciprocal(rec, den)
                res = finp.tile([P, NT, OUT_DIM], F32)
                nc.vector.tensor_tensor(
                    res,
                    accs[:, :, 0:OUT_DIM],
                    rec[:, :, None].to_broadcast([P, NT, OUT_DIM]),
                    mybir.AluOpType.mult,
                )
                nc.scalar.dma_start(out.rearrange("(t p) d -> p t d", p=P), res)

    nc.finalize()
    return nc


def _pack_kmajor(mat, ncols):
    """[256, ncols] f32 -> [128, 2*ncols] bf16 with row (ko*128+ki) at
    [ki, ko*ncols : (ko+1)*ncols] (one contiguous run per partition)."""
    b = mat.astype(ml_dtypes.bfloat16)
    return np.ascontiguousarray(
        b.reshape(KCH, P, ncols).transpose(1, 0, 2).reshape(P, KCH * ncols)
    )


def make_in_maps(cfg, struct, data, h, W_fc, a_attn):
    NPC, NPAD, NF = cfg.NPC, cfg.NPAD, cfg.NF
    h = np.asarray(h, np.float32)
    W_fc = np.asarray(W_fc, np.float32)
    a_attn = np.asarray(a_attn, np.float32)

    w_s = W_fc.T @ a_attn[:OUT_DIM]     # [256]
    w_d = W_fc.T @ a_attn[OUT_DIM:]
    W_aug = np.concatenate(
        [W_fc.T, w_s[:, None], w_d[:, None]], axis=1
    ).astype(np.float32)                # [256, 130]
    W_augb = _pack_kmajor(W_aug, 130)

    hT_full = np.zeros((IN_DIM, NF), np.float32)
    hT_full[:, : cfg.N] = h.T
    hTb = _pack_kmajor(hT_full, NF)

    asrc_b = np.ascontiguousarray(
        np.tile(a_attn[:OUT_DIM].astype(ml_dtypes.bfloat16)[None, :], (P, 1))
    )
    ident = np.eye(P, dtype=ml_dtypes.bfloat16)

    in_maps = []
    for c in range(NC):
        hTo = np.zeros((IN_DIM, NPAD), np.float32)
        hTo[:, :NPC] = hT_full[:, c * NPC:(c + 1) * NPC]
        gidx = np.stack([data[(c, 0)]["gidx"], data[(c, 1)]["gidx"]])
        gmask = np.stack([data[(c, 0)]["gmask"], data[(c, 1)]["gmask"]])
        mscat = np.stack([data[(c, 0)]["mscat"], data[(c, 1)]["mscat"]])
        zdscat = np.stack([data[(c, 0)]["zdscat"], data[(c, 1)]["zdscat"]])
        in_maps.append({
            "hTb": hTb,
            "hTownb": _pack_kmajor(hTo, NPAD),
            "W_augb": W_augb,
            "asrcb": asrc_b,
            "ident": ident,
            "gidx": np.ascontiguousarray(gidx),
            "gmask": np.ascontiguousarray(gmask),
            "mscat": np.ascontiguousarray(mscat),
            "zdscat": np.ascontiguousarray(zdscat),
        })
    return in_maps


def run(h, src, dst, W_fc, a_attn, n_nodes=None, n_edges=None, trace=False):
    h = np.asarray(h, np.float32)
    cfg = Cfg(
        n_nodes if n_nodes is not None else h.shape[0],
        n_edges if n_edges is not None else np.asarray(src).shape[0],
    )
    struct, data = host_prep(cfg, src, dst)
    nc = build_program(cfg, struct)
    in_maps = make_in_maps(cfg, struct, data, h, W_fc, a_attn)
    results = run_bass_kernel_spmd(
        nc, in_maps, core_ids=list(range(NC)), trace=trace
    )
    outs = [r["out"] for r in results.results]
    full = np.concatenate([o[: cfg.NPC] for o in outs], axis=0).astype(np.float32)
    return full, results


def kernel(h, src, dst, W_fc, a_attn):
    full, _ = run(h, src, dst, W_fc, a_attn)
    return full


# revision 37
# speedup vs baseline: 225.5576x; 166.0191x over previous
"""GAT layer (gnn_message_passing) on 8 Trainium2 NeuronCores.

Strategy (replicated z, zero collectives, dst-owner edge sharding):
  - Every core computes the FULL z = h @ W.T table (h shipped bf16 in a
    k-major packed layout; ~25.6MB load + bf16 matmuls) and writes it to
    a local DRAM table of pure-z 256B rows (identity node->row mapping).
    This replaces a sharded-z AllGather (was 46% of the critical path)
    and removes every inter-core sync point.
  - The table is split in two half tensors at node 25088 so int16 gather
    indices fit and half-0 edge processing can start while half-1 of the
    z table is still being computed (emission interleaves the phases).
  - Edges are bucketed by the owner of dst (6250 nodes/core), so segment
    softmax is fully local.  Softmax uses the no-max-subtraction form.
  - Per (core, half) the owned nodes are degree-sorted into a dst-major
    slot layout: partition p = node slot, free column = edge rank.
    dma_gather fetches z[src] rows straight into that layout, in ranges
    of up to GCOLS columns (one gather feeds several W-run chunks; all
    elementwise work runs range-wide to amortize per-op overheads).
  - zs[src] per edge is recomputed on the vector engine from the gathered
    bf16 z rows (dot with a_src along the free axis) instead of being
    stored in the table -- keeps gather rows at 256B and the gather
    descriptor-bound cost at its floor.
  - zd for owned nodes comes from a small per-core matmul over the core's
    own h slice (w_d column only), permuted to slot order via a
    dma_scatter_add round trip through DRAM (zdperm).
  - Edge scores use an additive -1e30 pad mask folded into zs, so
    exp() gives exact zeros at padding with no separate mask multiply.
  - The weighted segment-sum runs on the TensorEngine as identity-lhsT
    matmul chains accumulating in PSUM; exz carries exm in column 128 so
    the same chain accumulates the softmax denominator for free.
  - Both halves scatter-add bf16 [agg|den] rows into the SAME accumulator
    block (scatter_add merges them); a final divide produces out.
  - Zero-in-degree nodes get a fake self-edge on the host so out == z.
  - DMA queues are routed deliberately (h loads + table writes on SP,
    gathers/scatters on the Pool SWDGE, metadata on Activation) because
    a DMA holds its issuing engine queue for the whole transfer.
"""

import numpy as np
import ml_dtypes

import concourse.bass as bass
import concourse.mybir as mybir
import concourse.tile as tile
from concourse import bacc
from concourse import library_config
from concourse.bass import ts
from concourse.bass_utils import run_bass_kernel_spmd

F32 = mybir.dt.float32
BF16 = mybir.dt.bfloat16
I16 = mybir.dt.int16

NC = 8          # cores
P = 128         # partitions
IN_DIM = 256
OUT_DIM = 128
KCH = IN_DIM // P       # 2 k-chunks for the feature matmuls
ROW = 128               # bf16 elems per table row (256 B, pure z)
ACC2 = 256              # bf16 elems per accumulator row (512 B)
GCOLS = 64              # gather columns per dma_gather call (2 chunks)
SCAT_ELEMS = 129        # f32 elems scattered per slot ([agg128 | den])
ZDP_STRIDE = 64         # f32 stride of the zd permute buffer (256 B)
CHUNK_COLS = 32         # max gather columns per chunk
HCHUNK = 6272           # h columns per phase-1 SBUF chunk (49 tiles; 4 chunks = table0)
GRP = 3                 # tiles per phase-1 PSUM group (3 PSUM banks)


class Cfg:
    def __init__(self, n_nodes, n_edges):
        assert n_nodes % NC == 0
        self.N = n_nodes
        self.E = n_edges
        self.NPC = n_nodes // NC                  # 6250 owned nodes/core
        self.NPAD = ((self.NPC + P - 1) // P) * P  # 6400
        self.NT = self.NPAD // P                   # 50 owned tiles
        self.NF = ((n_nodes + P - 1) // P) * P     # 50048 full-table rows
        self.NTF = self.NF // P                    # 391 full tiles
        # table halves split at a tile boundary so int16 gather indices fit
        self.HALF = (self.NTF // 2 + 1) * P        # 25088
        self.HB = [0, self.HALF]
        self.HROWS = [self.HALF, self.NF - self.HALF]
        assert max(self.HROWS) < 32768


def _wrap16(flat, dtype=np.int16):
    """flat[i] -> [128, len/16] with flat[i] at [i%16, i//16], replicated x8."""
    n = flat.shape[0]
    assert n % 16 == 0
    w = flat.reshape(n // 16, 16).T.astype(dtype)  # [16, n/16]
    return np.tile(w, (8, 1))


def host_prep(cfg, src, dst):
    """Build the common tile structure + per-core index/mask arrays."""
    N, NPC, NPAD, HALF = cfg.N, cfg.NPC, cfg.NPAD, cfg.HALF
    src = np.asarray(src, np.int64).copy()
    dst = np.asarray(dst, np.int64).copy()

    # fake self-edges for isolated (zero in-degree) nodes -> out == z exactly
    deg_tot = np.bincount(dst, minlength=N)
    iso = np.nonzero(deg_tot == 0)[0]
    if iso.size:
        src = np.concatenate([src, iso])
        dst = np.concatenate([dst, iso])

    owner = dst // NPC
    halves = (src >= HALF).astype(np.int64)
    table_local = src - halves * HALF            # identity node layout
    assert table_local.max() < 32768 and table_local.min() >= 0

    per = {}
    deg_sorted_all = []
    for c in range(NC):
        for h in (0, 1):
            m = (owner == c) & (halves == h)
            es = table_local[m]               # gather index of each edge
            ed = dst[m] - c * NPC             # local dst node
            deg = np.bincount(ed, minlength=NPAD)
            order = np.argsort(-deg, kind="stable")  # node_of_slot [NPAD]
            sL = np.empty(NPAD, np.int64)
            sL[order] = np.arange(NPAD)              # slot of node
            per[(c, h)] = dict(es=es, ed=ed, deg=deg, order=order, sL=sL)
            deg_sorted_all.append(deg[order])

    # common tile widths
    NT = cfg.NT
    W = np.zeros(NT, np.int64)
    for dsrt in deg_sorted_all:
        W = np.maximum(W, dsrt[::P][:NT])
    NTp = int(np.nonzero(W > 0)[0][-1]) + 1 if (W > 0).any() else 0
    W = W[:NTp]
    colstart = np.concatenate([[0], np.cumsum(W)]).astype(np.int64)
    C = int(colstart[-1])
    CP = C  # gather cols total per half

    # chunks: runs of equal W, capped at CHUNK_COLS columns
    chunks = []  # (t0, nt, W)
    t = 0
    while t < NTp:
        w = int(W[t])
        nt = 1
        while (
            t + nt < NTp
            and int(W[t + nt]) == w
            and (nt + 1) * w <= CHUNK_COLS
        ):
            nt += 1
        chunks.append((t, nt, w))
        t += nt

    # per-(core,half) flat arrays
    data = {}
    for c in range(NC):
        for h in (0, 1):
            d = per[(c, h)]
            es, ed, deg, order, sL = (
                d["es"], d["ed"], d["deg"], d["order"], d["sL"],
            )
            slot = sL[ed]
            tile_of = slot // P
            part_of = slot % P
            # rank of each edge within its dst node
            o = np.argsort(slot, kind="stable")
            slot_s = slot[o]
            es_s = es[o]
            tile_s = tile_of[o]
            part_s = part_of[o]
            counts = np.bincount(slot_s, minlength=NPAD)
            starts = np.concatenate([[0], np.cumsum(counts)])[:-1]
            rank = np.arange(slot_s.size) - starts[slot_s]
            keep = tile_s < NTp
            assert keep.all(), "edge landed outside processed tiles"
            cglob = colstart[tile_s] + rank
            assert (rank < W[tile_s]).all()
            pos = cglob * P + part_s

            flat_idx = np.zeros(CP * P, np.int16)
            flat_idx[pos] = es_s.astype(np.int16)
            # additive mask: 0 at real edges, -1e30 at padding (exp -> 0)
            mask = np.full((P, CP), -1e30, np.float32)
            mask[part_s, cglob] = 0.0

            data[(c, h)] = dict(
                gidx=_wrap16(flat_idx),
                gmask=mask,
                mscat=_wrap16(order[: NTp * P].astype(np.int16)),
                zdscat=_wrap16(sL.astype(np.int16)),
            )

    # pack consecutive chunks into gather ranges of <= GCOLS columns
    ranges = []  # (col0, ccr, [(t0, ntc, w, off), ...])
    cur = None
    for (t0, nt_, w) in chunks:
        cc = nt_ * w
        col0 = int(colstart[t0])
        if cur is not None and (col0 + cc - cur[0]) <= GCOLS:
            cur[2].append((t0, nt_, w, col0 - cur[0]))
            cur[1] = col0 + cc - cur[0]
        else:
            cur = [col0, cc, [(t0, nt_, w, 0)]]
            ranges.append(cur)
    ranges = [(r[0], r[1], r[2]) for r in ranges]

    struct = dict(W=W, NTp=NTp, colstart=colstart, C=C, chunks=chunks,
                  ranges=ranges)
    return struct, data


def build_program(cfg, struct, stage=5, sub=4):
    NPAD, NT, NF, NTF = cfg.NPAD, cfg.NT, cfg.NF, cfg.NTF
    NTp, C = struct["NTp"], struct["C"]
    W, colstart, chunks = struct["W"], struct["colstart"], struct["chunks"]
    ranges = struct["ranges"]

    nc = bacc.Bacc(
        "TRN2", target_bir_lowering=False, debug=False, num_devices=NC
    )

    # I/O
    hTb = nc.dram_tensor("hTb", [P, KCH * NF], BF16, kind="ExternalInput").ap()
    hTownb = nc.dram_tensor("hTownb", [P, KCH * NPAD], BF16, kind="ExternalInput").ap()
    W_augb = nc.dram_tensor("W_augb", [P, KCH * 130], BF16, kind="ExternalInput").ap()
    asrcb = nc.dram_tensor("asrcb", [P, 128], BF16, kind="ExternalInput").ap()
    ident_in = nc.dram_tensor("ident", [P, P], BF16, kind="ExternalInput").ap()
    gidx_in = nc.dram_tensor("gidx", [2, P, C * 8], I16, kind="ExternalInput").ap()
    gmask_in = nc.dram_tensor("gmask", [2, P, C], F32, kind="ExternalInput").ap()
    mscat_in = nc.dram_tensor("mscat", [2, P, NTp * 8], I16, kind="ExternalInput").ap()
    zdscat_in = nc.dram_tensor("zdscat", [2, P, NPAD // 16], I16, kind="ExternalInput").ap()

    out = nc.dram_tensor("out", [NPAD, OUT_DIM], F32, kind="ExternalOutput").ap()
    acc = nc.dram_tensor("acc", [NPAD, ACC2], BF16, kind="ExternalOutput").ap()
    zdperm = nc.dram_tensor("zdperm", [2 * NPAD, ZDP_STRIDE], F32, kind="ExternalOutput").ap()

    # two half tables so half-0 gathers need not wait for half-1 writes
    tables = [
        nc.dram_tensor(f"table{h}", [cfg.HROWS[h], ROW], BF16, kind="Internal").ap()
        for h in (0, 1)
    ]

    nc.gpsimd.load_library(library_config.mlp)

    with tile.TileContext(nc) as tc:
        with tc.tile_pool(name="const", bufs=1) as constp:
            ident = constp.tile([P, P], BF16)
            nc.scalar.dma_start(ident, ident_in)
            asrc = constp.tile([P, 128], BF16)
            nc.scalar.dma_start(asrc, asrcb)
            wsb = constp.tile([P, KCH, 130], BF16)
            nc.scalar.dma_start(wsb, W_augb.rearrange("p (ko m) -> p ko m", ko=KCH))

            # edge metadata early so phase-C gathers aren't gated on DMA order
            gidx_sb = []
            gmask_sb = []
            mscat_sb = []
            for h in (0, 1) if stage >= 4 else ():
                g = constp.tile([P, C * 8], I16, tag=f"gidx{h}")
                nc.scalar.dma_start(g, gidx_in[h])
                gidx_sb.append(g)
                m = constp.tile([P, C], F32, tag=f"gmask{h}")
                nc.scalar.dma_start(m, gmask_in[h])
                gmask_sb.append(m)
                s = constp.tile([P, NTp * 8], I16, tag=f"mscat{h}")
                nc.scalar.dma_start(s, mscat_in[h])
                mscat_sb.append(s)

            # phase-C pools open FIRST so their SBUF zones are disjoint from
            # phase-B's (address reuse would turn gathers' G-tile writes into
            # false WAR deps on phase-B's last matmul).  Emission interleaves
            # the phases: B chunks covering table0 -> C half 0 -> remaining B
            # chunks -> C half 1, so half-0 edge work overlaps half-1 z work.
            with (
                tc.tile_pool(name="meta", bufs=1) as metap,
                tc.tile_pool(name="gbuf", bufs=2) as gbuf,
                tc.tile_pool(name="ebig", bufs=2) as ebig,
                tc.tile_pool(name="ebuf", bufs=2) as ebuf,
                tc.tile_pool(name="aggb", bufs=1) as aggb,
                tc.tile_pool(name="ps2", bufs=2, space="PSUM") as ps2,
                tc.tile_pool(name="ph1h", bufs=2) as ph1h,
                tc.tile_pool(name="ph1r", bufs=2) as ph1r,
                tc.tile_pool(name="ph1ps", bufs=2, space="PSUM") as ph1ps,
            ):

                def emit_b_chunk(c0):
                    csz = min(HCHUNK, NF - c0)
                    hsb = ph1h.tile([P, KCH, HCHUNK], BF16, tag="hsb")
                    nc.sync.dma_start(
                        hsb[:, :, 0:csz],
                        hTb.rearrange("p (ko n) -> p ko n", ko=KCH)[:, :, c0:c0 + csz],
                    )
                    rows = ph1r.tile([P, HCHUNK // P, ROW], BF16, tag="rows")
                    for gi, g0 in enumerate(range(0, csz, GRP * P)):
                        gsz = min(GRP * P, csz - g0)
                        ntg = gsz // P
                        ps3 = ph1ps.tile([P, GRP, 512], F32)
                        for i in range(ntg):
                            for k in range(KCH):
                                nc.tensor.matmul(
                                    ps3[:, i, 0:128],
                                    lhsT=hsb[:, k, g0 + i * P : g0 + (i + 1) * P],
                                    rhs=wsb[:, k, 0:128],
                                    start=(k == 0),
                                    stop=(k == KCH - 1),
                                )
                        t0 = g0 // P
                        nc.scalar.copy(rows[:, t0 : t0 + ntg, :], ps3[:, 0:ntg, 0:128])
                    # one batched table write per chunk, split at the half
                    # boundary so each half-table's writes finish ASAP
                    spans = []
                    if c0 < cfg.HALF:
                        hi = min(c0 + csz, cfg.HALF)
                        spans.append((0, c0, hi - c0))
                    if c0 + csz > cfg.HALF:
                        lo = max(c0, cfg.HALF)
                        spans.append((1, lo - cfg.HALF, c0 + csz - lo))
                    for (hh, r0, rn) in spans:
                        toff = (cfg.HB[hh] + r0) - c0  # offset in rows tile
                        nc.sync.dma_start(
                            tables[hh][r0 : r0 + rn, :].rearrange(
                                "(t p) d -> p t d", p=P
                            ),
                            rows[:, toff // P : (toff + rn) // P, :],
                        )

                def emit_mini():
                    """Own-node zd (cols of the augmented matmul) + slot
                    permute.  Runs between table0 chunks and half-0 edge work
                    so its matmuls don't block phase-B's PE queue."""
                    assert HCHUNK == NT * P == NPAD
                    hto = ph1h.tile([P, KCH, HCHUNK], BF16, tag="hsb")
                    nc.gpsimd.dma_start(
                        hto,
                        hTownb.rearrange("p (ko n) -> p ko n", ko=KCH),
                    )
                    psz = ps2.tile([P, SCAT_ELEMS], F32, tag="aggps")
                    for t in range(NT):
                        for k in range(KCH):
                            nc.tensor.matmul(
                                psz[:, t : t + 1],
                                lhsT=hto[:, k, ts(t, P)],
                                rhs=wsb[:, k, 129:130],
                                start=(k == 0),
                                stop=(k == KCH - 1),
                            )
                    zd_sb = metap.tile([P, NT, 1], F32, tag="zdsb")
                    nc.vector.tensor_copy(zd_sb[:, :, 0], psz[:, 0:NT])
                    for h in (0, 1) if stage >= 2 else ():
                        zdsc = metap.tile([P, NPAD // 16], I16, tag=f"zdsc{h}")
                        nc.scalar.dma_start(zdsc, zdscat_in[h])
                        nc.gpsimd.dma_scatter_add(
                            out_ap=zdperm[h * NPAD:(h + 1) * NPAD, 0:1],
                            in_ap=zd_sb,
                            idxs_ap=zdsc,
                            num_idxs=NPAD,
                            num_idxs_reg=NPAD,
                            elem_size=1,
                            elem_step=ZDP_STRIDE,
                            single_packet=NPAD <= 1024,
                        )

                def emit_c_half(h):
                    z = metap.tile([P, NT, 1], F32, tag=f"zds{h}")
                    nc.scalar.dma_start(
                        z,
                        zdperm[h * NPAD:(h + 1) * NPAD, :]
                        .rearrange("(t p) d -> p t d", p=P)[:, :, 0:1],
                    )
                    if stage < 5:
                        return
                    agg = aggb.tile([P, NTp, SCAT_ELEMS], BF16, tag=f"agg{h}")
                    for (col0, ccr, rchunks) in ranges:
                        G = gbuf.tile([P, GCOLS, ROW], BF16, tag="G")
                        nc.gpsimd.dma_gather(
                            out_ap=G[:, 0:ccr, :],
                            in_ap=tables[h],
                            idxs_ap=gidx_sb[h][:, col0 * 8:(col0 + ccr) * 8],
                            num_idxs=ccr * P,
                            num_idxs_reg=ccr * P,
                            elem_size=ROW,
                            single_packet=ccr * P <= 1024,
                        )
                        if sub < 2:
                            continue
                        # zs[src] = gathered z . a_src (free-axis dot),
                        # range-wide ops amortize per-op overheads
                        zsp = ebig.tile([P, GCOLS, SCAT_ELEMS], BF16, tag="exz")
                        nc.vector.tensor_tensor(
                            zsp[:, 0:ccr, 0:ROW],
                            G[:, 0:ccr, :],
                            asrc[:, None, :].to_broadcast([P, ccr, ROW]),
                            mybir.AluOpType.mult,
                        )
                        zsc = ebuf.tile([P, GCOLS], F32, tag="zsc")
                        nc.vector.tensor_reduce(
                            zsc[:, 0:ccr],
                            zsp[:, 0:ccr, 0:ROW],
                            mybir.AxisListType.X,
                            mybir.AluOpType.add,
                        )
                        # scores: zsc + additive pad mask, + zd (per chunk),
                        # then range-wide leaky + exp
                        score = ebuf.tile([P, GCOLS], F32, tag="score")
                        nc.vector.tensor_tensor(
                            score[:, 0:ccr], zsc[:, 0:ccr],
                            gmask_sb[h][:, col0:col0 + ccr],
                            mybir.AluOpType.add,
                        )
                        for (t0, ntc, w, off) in rchunks:
                            sc = score[:, off:off + ntc * w].rearrange(
                                "p (t w) -> p t w", w=w
                            )
                            nc.vector.tensor_tensor(
                                sc,
                                sc,
                                z[:, t0:t0 + ntc, :].to_broadcast([P, ntc, w]),
                                mybir.AluOpType.add,
                            )
                        exf = ebuf.tile([P, GCOLS], F32, tag="exf")
                        nc.vector.scalar_tensor_tensor(
                            exf[:, 0:ccr], score[:, 0:ccr], 0.01, score[:, 0:ccr],
                            op0=mybir.AluOpType.mult, op1=mybir.AluOpType.max,
                        )
                        exm = ebuf.tile([P, GCOLS], BF16, tag="exm")
                        nc.scalar.activation(
                            exm[:, 0:ccr], exf[:, 0:ccr],
                            mybir.ActivationFunctionType.Exp,
                        )
                        # exz rows [z*exm | exm]: col 128 makes the PE
                        # chain accumulate the denominator for free
                        exz = ebig.tile([P, GCOLS, SCAT_ELEMS], BF16, tag="exz")
                        nc.vector.tensor_tensor(
                            exz[:, 0:ccr, 0:ROW],
                            G[:, 0:ccr, :],
                            exm[:, 0:ccr, None].to_broadcast([P, ccr, ROW]),
                            mybir.AluOpType.mult,
                        )
                        nc.vector.tensor_copy(exz[:, 0:ccr, ROW], exm[:, 0:ccr])
                        if sub < 3:
                            continue
                        for (t0, ntc, w, off) in rchunks:
                            for ti in range(ntc):
                                ps = ps2.tile([P, SCAT_ELEMS], F32, tag="aggps")
                                for r in range(w):
                                    nc.tensor.matmul(
                                        ps,
                                        lhsT=ident,
                                        rhs=exz[:, off + ti * w + r, :],
                                        start=(r == 0),
                                        stop=(r == w - 1),
                                    )
                                nc.scalar.copy(agg[:, t0 + ti, 0:SCAT_ELEMS], ps)
                    if sub < 4:
                        return
                    nc.gpsimd.dma_scatter_add(
                        out_ap=acc[:, 0:SCAT_ELEMS],
                        in_ap=agg,
                        idxs_ap=mscat_sb[h],
                        num_idxs=NTp * P,
                        num_idxs_reg=NTp * P,
                        elem_size=SCAT_ELEMS,
                        elem_step=ACC2,
                        single_packet=NTp * P <= 1024,
                    )

                b_chunks = list(range(0, NF, HCHUNK)) if stage >= 3 else []
                b_first = [c for c in b_chunks if c < cfg.HALF]
                b_rest = [c for c in b_chunks if c >= cfg.HALF]
                for ci, c0 in enumerate(b_first):
                    emit_b_chunk(c0)
                    if ci == 0:
                        emit_mini()
                if stage >= 4:
                    emit_c_half(0)
                for c0 in b_rest:
                    emit_b_chunk(c0)
                if stage >= 4:
                    emit_c_half(1)

            # ---------------- phase D: divide + output ----------------
            with tc.tile_pool(name="fin", bufs=1) as finp:
                accs = finp.tile([P, NT, SCAT_ELEMS], BF16)
                av = acc.rearrange("(t p) d -> p t d", p=P)
                nc.sync.dma_start(accs, av[:, :, 0:SCAT_ELEMS])
                den = finp.tile([P, NT], F32)
                nc.vector.tensor_copy(den, accs[:, :, 128])
                nc.vector.tensor_scalar(
                    den, den, 1e-30, None, mybir.AluOpType.max
                )
                rec = finp.tile([P, NT], F32)
                nc.vector.reciprocal(rec, den)
                res = finp.tile([P, NT, OUT_DIM], F32)
                nc.vector.tensor_tensor(
                    res,
                    accs[:, :, 0:OUT_DIM],
                    rec[:, :, None].to_broadcast([P, NT, OUT_DIM]),
                    mybir.AluOpType.mult,
                )
                nc.scalar.dma_start(out.rearrange("(t p) d -> p t d", p=P), res)

    nc.finalize()
    return nc


def _pack_kmajor(mat, ncols):
    """[256, ncols] f32 -> [128, 2*ncols] bf16 with row (ko*128+ki) at
    [ki, ko*ncols : (ko+1)*ncols] (one contiguous run per partition)."""
    b = mat.astype(ml_dtypes.bfloat16)
    return np.ascontiguousarray(
        b.reshape(KCH, P, ncols).transpose(1, 0, 2).reshape(P, KCH * ncols)
    )


def make_in_maps(cfg, struct, data, h, W_fc, a_attn):
    NPC, NPAD, NF = cfg.NPC, cfg.NPAD, cfg.NF
    h = np.asarray(h, np.float32)
    W_fc = np.asarray(W_fc, np.float32)
    a_attn = np.asarray(a_attn, np.float32)

    w_s = W_fc.T @ a_attn[:OUT_DIM]     # [256]
    w_d = W_fc.T @ a_attn[OUT_DIM:]
    W_aug = np.concatenate(
        [W_fc.T, w_s[:, None], w_d[:, None]], axis=1
    ).astype(np.float32)                # [256, 130]
    W_augb = _pack_kmajor(W_aug, 130)

    hT_full = np.zeros((IN_DIM, NF), np.float32)
    hT_full[:, : cfg.N] = h.T
    hTb = _pack_kmajor(hT_full, NF)

    asrc_b = np.ascontiguousarray(
        np.tile(a_attn[:OUT_DIM].astype(ml_dtypes.bfloat16)[None, :], (P, 1))
    )
    ident = np.eye(P, dtype=ml_dtypes.bfloat16)

    in_maps = []
    for c in range(NC):
        hTo = np.zeros((IN_DIM, NPAD), np.float32)
        hTo[:, :NPC] = hT_full[:, c * NPC:(c + 1) * NPC]
        gidx = np.stack([data[(c, 0)]["gidx"], data[(c, 1)]["gidx"]])
        gmask = np.stack([data[(c, 0)]["gmask"], data[(c, 1)]["gmask"]])
        mscat = np.stack([data[(c, 0)]["mscat"], data[(c, 1)]["mscat"]])
        zdscat = np.stack([data[(c, 0)]["zdscat"], data[(c, 1)]["zdscat"]])
        in_maps.append({
            "hTb": hTb,
            "hTownb": _pack_kmajor(hTo, NPAD),
            "W_augb": W_augb,
            "asrcb": asrc_b,
            "ident": ident,
            "gidx": np.ascontiguousarray(gidx),
            "gmask": np.ascontiguousarray(gmask),
            "mscat": np.ascontiguousarray(mscat),
            "zdscat": np.ascontiguousarray(zdscat),
        })
    return in_maps


def run(h, src, dst, W_fc, a_attn, n_nodes=None, n_edges=None, trace=False):
    h = np.asarray(h, np.float32)
    cfg = Cfg(
        n_nodes if n_nodes is not None else h.shape[0],
        n_edges if n_edges is not None else np.asarray(src).shape[0],
    )
    struct, data = host_prep(cfg, src, dst)
    nc = build_program(cfg, struct)
    in_maps = make_in_maps(cfg, struct, data, h, W_fc, a_attn)
    results = run_bass_kernel_spmd(
        nc, in_maps, core_ids=list(range(NC)), trace=trace
    )
    outs = [r["out"] for r in results.results]
    full = np.concatenate([o[: cfg.NPC] for o in outs], axis=0).astype(np.float32)
    return full, results


def kernel(h, src, dst, W_fc, a_attn):
    full, _ = run(h, src, dst, W_fc, a_attn)
    return full
